# revision 2
# baseline (speedup 1.0000x reference)
"""DistMult scoring kernel v2 for Trainium2 (8 NeuronCores, Bass/Tile).

reference computation:
    rel = rel_embeds[rel_ids]                      # [B, D] gather
    scores = sum(head * rel * tail, axis=-1)       # [B]
    pos = min(scores[:n_pos], upper_bound)
    neg = max(scores[n_pos:], lower_bound)
    out = sigmoid(concat(pos, neg))

Design: host-sorts rows by rel_id and pads each rel group to a multiple of
SUB=256 rows, so every 256-row sub-block shares ONE relation. Rows are
streamed transposed (d on partitions) in fp8. The device then:
  - gathers each sub-block's rel vector (64 descriptors per 32-block group)
  - transposes them on the PE and scatters into diagonal stationary slots
  - computes q = h*t elementwise (DVE/gpsimd split)
  - performs rel-multiply AND the d-reduction as PE matmuls:
      stationary = rel vector in column j (zeros elsewhere),
      32 blocks accumulate into one [32, 512] PSUM tile (partition = block)
  - clamp + sigmoid tail, DMA out
Per-row bounds are pre-permuted on host with +/-inf padding; the output is
inverse-permuted on host.
"""

import sys

for _p in ("/opt/trn_rl_repo",):
    if _p not in sys.path:
        sys.path.insert(0, _p)

import numpy as np
import ml_dtypes

import concourse.bacc as bacc
import concourse.bass as bass
import concourse.mybir as mybir
import concourse.tile as tile
from concourse.bass_utils import run_bass_kernel_spmd

N_POS = 131072
N_NEG = 393216
B = N_POS + N_NEG  # 524288
D = 256
NUM_REL = 500
NCORES = 8
SUB = 256          # rows per rel-pure sub-block
BS = 512           # rows per block (2 subs): DMA/TT1/moving granularity
GBLK = 32          # blocks per psum group

F8 = ml_dtypes.float8_e4m3
BF16 = ml_dtypes.bfloat16

f32 = mybir.dt.float32
bf16 = mybir.dt.bfloat16
fp8 = mybir.dt.float8e4
i32 = mybir.dt.int32
MULT = mybir.AluOpType.mult
MIN = mybir.AluOpType.min
MAX = mybir.AluOpType.max


# ---------------------------------------------------------------- host plan

def plan(rel_ids):
    """Sort rows by rel, pad groups to SUB multiples, split across cores."""
    rel_ids = np.asarray(rel_ids).astype(np.int64)
    order = np.argsort(rel_ids, kind="stable")
    counts = np.bincount(rel_ids, minlength=NUM_REL)
    subs_per_rel = (counts + SUB - 1) // SUB
    total_subs = int(subs_per_rel.sum())
    pad_subs = (-total_subs) % (2 * NCORES)
    s_all = total_subs + pad_subs

    starts = np.concatenate([[0], np.cumsum(counts)])
    pstarts = np.concatenate([[0], np.cumsum(subs_per_rel * SUB)])
    r_all = s_all * SUB
    padded_orig = np.full(r_all, -1, np.int64)
    sub_rel = np.zeros(s_all, np.int32)
    for k in range(NUM_REL):
        n = int(counts[k])
        if n == 0:
            continue
        p0 = int(pstarts[k])
        padded_orig[p0 : p0 + n] = order[starts[k] : starts[k] + n]
        sub_rel[p0 // SUB : p0 // SUB + int(subs_per_rel[k])] = k

    s_core = s_all // NCORES
    nblk = s_core // 2
    ngrp = (nblk + GBLK - 1) // GBLK
    return padded_orig, sub_rel, s_core, nblk, ngrp


# ---------------------------------------------------------------- device

def build_program(nblk, ngrp, debug=False):
    r_core = nblk * BS
    r_rect = ngrp * GBLK * BS

    nc = bacc.Bacc(
        "TRN2", target_bir_lowering=False, debug=False, num_devices=NCORES
    )
    npair = (ngrp + 1) // 2
    hT = nc.declare_dram_parameter("hT", [2, 128, r_core], fp8, isOutput=False)
    tT = nc.declare_dram_parameter("tT", [2, 128, r_core], fp8, isOutput=False)
    table = nc.declare_dram_parameter("table", [NUM_REL, D], bf16, isOutput=False)
    ids = nc.declare_dram_parameter("ids", [128, npair], i32, isOutput=False)
    ident4 = nc.declare_dram_parameter("ident4", [128, 128], bf16, isOutput=False)
    ub = nc.declare_dram_parameter("ub", [r_rect], f32, isOutput=False)
    lb = nc.declare_dram_parameter("lb", [r_rect], f32, isOutput=False)
    out = nc.declare_dram_parameter("out", [r_rect], f32, isOutput=True)
    if debug:
        dbg_st = nc.declare_dram_parameter("dbg_st", [128, 2, 2, GBLK * GBLK], f32, isOutput=True)
        dbg_q = nc.declare_dram_parameter("dbg_q", [128, 2, BS], f32, isOutput=True)
        dbg_s = nc.declare_dram_parameter("dbg_s", [GBLK, BS], f32, isOutput=True)
        dbg_rt = nc.declare_dram_parameter("dbg_rt", [128, D], f32, isOutput=True)

    hT_v = hT[:].rearrange("c p r -> p c r")
    tT_v = tT[:].rearrange("c p r -> p c r")
    # rect layouts indexed (j, g, c)
    ub_v = ub[:].rearrange("(j g c) -> j g c", j=GBLK, g=ngrp, c=BS)
    lb_v = lb[:].rearrange("(j g c) -> j g c", j=GBLK, g=ngrp, c=BS)
    out_v = out[:].rearrange("(j g c) -> j g c", j=GBLK, g=ngrp, c=BS)

    with tile.TileContext(nc) as tc:
        with (
            tc.tile_pool(name="io", bufs=1) as io,
            tc.tile_pool(name="hp", bufs=4) as hp,
            tc.tile_pool(name="tp", bufs=4) as tpo,
            tc.tile_pool(name="qp", bufs=4) as qp,
            tc.tile_pool(name="bp", bufs=4) as bp,
            tc.tile_pool(name="sp", bufs=4) as sp,
            tc.psum_pool(name="ps", bufs=2) as pspool,
            tc.psum_pool(name="tps", bufs=2) as tpspool,
        ):
            ids_t = io.tile([128, npair], i32)
            nc.gpsimd.dma_start(out=ids_t[:], in_=ids[:])
            ident_t = io.tile([128, 128], bf16)
            nc.sync.dma_start(out=ident_t[:], in_=ident4[:])

            # staging: [128, ch, u, slot*col] fp8; two fixed buffers
            stagings = [io.tile([128, 2, 2, GBLK * GBLK], fp8, name=f"staging{i}") for i in range(2)]
            for st in stagings:
                nc.vector.memset(st[:], 0.0)
            # pair gather: 128 rel rows; partition p = (g%2)*64 + u*32 + j
            reltiles = [io.tile([128, D], bf16, name=f"reltile{i}") for i in range(2)]

            def stage_pair(m):
                """Gather + stage groups 2m and 2m+1 (if present)."""
                rt = reltiles[m % 2]
                nc.gpsimd.indirect_dma_start(
                    out=rt[:],
                    out_offset=None,
                    in_=table[:],
                    in_offset=bass.IndirectOffsetOnAxis(
                        ap=ids_t[:, m : m + 1], axis=0
                    ),
                )
                for ch in range(2):
                    tpsum = tpspool.tile([128, 128], bf16, tag="tp")
                    nc.tensor.transpose(
                        tpsum[:], rt[:, ch * 128 : (ch + 1) * 128], ident_t[:]
                    )
                    for half in range(2):
                        g = 2 * m + half
                        if g >= ngrp:
                            break
                        st = stagings[g % 2]
                        for u in range(2):
                            base = half * 64 + u * 32
                            # scatter columns into diagonal slots (stride GBLK+1)
                            nc.vector.tensor_copy(
                                out=st[:, ch, u, 0 : GBLK * GBLK : GBLK + 1],
                                in_=tpsum[:, base : base + GBLK],
                            )

            stage_pair(0)

            for g in range(ngrp):
                nblk_g = min(GBLK, nblk - g * GBLK)
                st = stagings[g % 2]
                pss = [pspool.tile([GBLK, SUB], f32, tag=f"ps{u}", name=f"ps{u}") for u in range(2)]
                # DMA units of 4 blocks (2KB runs per partition/chunk)
                j = 0
                tt_unit = 0
                while j < nblk_g:
                    nu = min(4, nblk_g - j)
                    b0 = g * GBLK + j
                    ht = hp.tile([128, 2, 4 * BS], fp8, tag="h")
                    tt = tpo.tile([128, 2, 4 * BS], fp8, tag="t")
                    nc.sync.dma_start(
                        out=ht[:, :, : nu * BS], in_=hT_v[:, :, b0 * BS : (b0 + nu) * BS]
                    )
                    nc.scalar.dma_start(
                        out=tt[:, :, : nu * BS], in_=tT_v[:, :, b0 * BS : (b0 + nu) * BS]
                    )
                    qt = qp.tile([128, 2, 4 * BS], fp8, tag="q")
                    # TT1 in units of 2 blocks, split DVE/gpsimd
                    k = 0
                    while k < nu:
                        nk = min(2, nu - k)
                        eng = nc.gpsimd if (tt_unit % 8 in (2, 5, 7)) else nc.vector
                        eng.tensor_tensor(
                            out=qt[:, :, k * BS : (k + nk) * BS],
                            in0=ht[:, :, k * BS : (k + nk) * BS],
                            in1=tt[:, :, k * BS : (k + nk) * BS],
                            op=MULT,
                        )
                        tt_unit += 1
                        k += nk
                    for k in range(nu):
                        jj = j + k
                        for u in range(2):
                            nc.tensor.matmul(
                                pss[u][:],
                                st[:, :, u, jj * GBLK : (jj + 1) * GBLK],
                                qt[:, :, k * BS + u * SUB : k * BS + (u + 1) * SUB],
                                start=(jj == 0),
                                stop=(jj == nblk_g - 1),
                                perf_mode=mybir.MatmulPerfMode.DoubleRow,
                            )
                    j += nu
                # tail for group g
                ubt = bp.tile([GBLK, BS], f32, tag="ub")
                lbt = bp.tile([GBLK, BS], f32, tag="lb")
                nc.sync.dma_start(out=ubt[:nblk_g], in_=ub_v[:nblk_g, g, :])
                nc.scalar.dma_start(out=lbt[:nblk_g], in_=lb_v[:nblk_g, g, :])
                c1 = sp.tile([GBLK, BS], f32, tag="c1")
                c2 = sp.tile([GBLK, BS], f32, tag="c2")
                for u in range(2):
                    cseg = slice(u * SUB, (u + 1) * SUB)
                    nc.vector.tensor_tensor(
                        out=c1[:nblk_g, cseg], in0=pss[u][:nblk_g],
                        in1=ubt[:nblk_g, cseg], op=MIN,
                    )
                nc.vector.tensor_tensor(
                    out=c2[:nblk_g], in0=c1[:nblk_g], in1=lbt[:nblk_g], op=MAX
                )
                sg = sp.tile([GBLK, BS], f32, tag="sg")
                nc.scalar.activation(
                    out=sg[:nblk_g], in_=c2[:nblk_g],
                    func=mybir.ActivationFunctionType.Sigmoid,
                )
                nc.scalar.dma_start(out=out_v[:nblk_g, g, :], in_=sg[:nblk_g])
                if debug and g == 0:
                    dst = sp.tile([128, 2, 2, GBLK * GBLK], f32, name="dst")
                    nc.vector.tensor_copy(out=dst[:], in_=st[:])
                    nc.sync.dma_start(out=dbg_st[:], in_=dst[:])
                    drt = sp.tile([128, D], f32, name="drt")
                    nc.vector.tensor_copy(out=drt[:], in_=reltiles[0][:])
                    nc.sync.dma_start(out=dbg_rt[:], in_=drt[:])
                    dsc = sp.tile([GBLK, BS], f32, name="dsc")
                    for u in range(2):
                        nc.vector.tensor_copy(out=dsc[:, u * SUB : (u + 1) * SUB], in_=pss[u][:])
                    nc.sync.dma_start(out=dbg_s[:], in_=dsc[:])
                if g % 2 == 1 and g + 1 < ngrp:
                    stage_pair((g + 1) // 2)

    nc.compile()
    return nc


# ---------------------------------------------------------------- host glue

def make_in_maps(inputs, padded_orig, sub_rel, s_core, nblk, ngrp):
    r_core = nblk * BS
    r_rect = ngrp * GBLK * BS

    head = np.asarray(inputs["head_embeds"], dtype=np.float32)
    tail = np.asarray(inputs["tail_embeds"], dtype=np.float32)
    lower = np.asarray(inputs["lower_bound"], dtype=np.float32)
    upper = np.asarray(inputs["upper_bound"], dtype=np.float32)
    table = np.asarray(inputs["rel_embeds"], dtype=np.float32).astype(BF16)

    h8 = head.astype(F8)
    t8 = tail.astype(F8)

    in_maps = []
    for c in range(NCORES):
        idx = padded_orig[c * r_core : (c + 1) * r_core]
        valid = idx >= 0
        hrows = np.zeros((r_core, D), dtype=F8)
        trows = np.zeros((r_core, D), dtype=F8)
        hrows[valid] = h8[idx[valid]]
        trows[valid] = t8[idx[valid]]
        hT = np.ascontiguousarray(hrows.reshape(r_core, 2, 128).transpose(1, 2, 0))
        tTm = np.ascontiguousarray(trows.reshape(r_core, 2, 128).transpose(1, 2, 0))

        ub_r = np.zeros(r_core, np.float32)
        lb_r = np.zeros(r_core, np.float32)
        mpos = valid & (idx < N_POS)
        mneg = idx >= N_POS
        ub_r[mpos] = upper[idx[mpos]]
        lb_r[mpos] = -np.inf
        ub_r[mneg] = np.inf
        lb_r[mneg] = lower[idx[mneg] - N_POS]

        def rectify(a):
            ap = np.zeros(r_rect, a.dtype)
            ap[:r_core] = a
            # (g, j, c) -> (j, g, c)
            return np.ascontiguousarray(
                ap.reshape(ngrp, GBLK, BS).transpose(1, 0, 2)
            ).reshape(-1)

        srel = sub_rel[c * s_core : (c + 1) * s_core]
        sp_ = np.zeros(ngrp * GBLK * 2, np.int32)
        sp_[:s_core] = srel
        srel_r = sp_.reshape(ngrp, GBLK, 2)  # (g, j, u)
        npair = (ngrp + 1) // 2
        ids_arr = np.zeros((128, npair), np.int32)
        for g in range(ngrp):
            # partition p = (g%2)*64 + u*32 + j
            ids_arr[(g % 2) * 64 : (g % 2) * 64 + 64, g // 2] = (
                srel_r[g].transpose(1, 0).reshape(64)
            )

        ident4 = np.eye(128, dtype=np.float32).astype(BF16)
        in_maps.append(
            {
                "hT": hT,
                "tT": tTm,
                "table": table,
                "ids": ids_arr,
                "ident4": ident4,
                "ub": rectify(ub_r),
                "lb": rectify(lb_r),
            }
        )
    return in_maps


def unpack_output(res, padded_orig, nblk, ngrp):
    r_core = nblk * BS
    out_full = np.empty(B, np.float32)
    for c in range(NCORES):
        rect = res.results[c]["out"].reshape(GBLK, ngrp, BS)
        flat = rect.transpose(1, 0, 2).reshape(-1)[:r_core]
        idx = padded_orig[c * r_core : (c + 1) * r_core]
        m = idx >= 0
        out_full[idx[m]] = flat[m]
    return out_full


def kernel(**inputs):
    padded_orig, sub_rel, s_core, nblk, ngrp = plan(inputs["rel_ids"])
    nc = build_program(nblk, ngrp)
    in_maps = make_in_maps(inputs, padded_orig, sub_rel, s_core, nblk, ngrp)
    res = run_bass_kernel_spmd(nc, in_maps, list(range(NCORES)))
    return unpack_output(res, padded_orig, nblk, ngrp)


# revision 3
# speedup vs baseline: 1.0417x; 1.0417x over previous
"""DistMult scoring kernel v2 for Trainium2 (8 NeuronCores, Bass/Tile).

reference computation:
    rel = rel_embeds[rel_ids]                      # [B, D] gather
    scores = sum(head * rel * tail, axis=-1)       # [B]
    pos = min(scores[:n_pos], upper_bound)
    neg = max(scores[n_pos:], lower_bound)
    out = sigmoid(concat(pos, neg))

Design: host-sorts rows by rel_id and pads each rel group to a multiple of
SUB=256 rows, so every 256-row sub-block shares ONE relation. Rows are
streamed transposed (d on partitions) in fp8. The device then:
  - gathers each sub-block's rel vector (64 descriptors per 32-block group)
  - transposes them on the PE and scatters into diagonal stationary slots
  - computes q = h*t elementwise (DVE/gpsimd split)
  - performs rel-multiply AND the d-reduction as PE matmuls:
      stationary = rel vector in column j (zeros elsewhere),
      32 blocks accumulate into one [32, 512] PSUM tile (partition = block)
  - clamp + sigmoid tail, DMA out
Per-row bounds are pre-permuted on host with +/-inf padding; the output is
inverse-permuted on host.
"""

import sys

for _p in ("/opt/trn_rl_repo",):
    if _p not in sys.path:
        sys.path.insert(0, _p)

import numpy as np
import ml_dtypes

import concourse.bacc as bacc
import concourse.bass as bass
import concourse.mybir as mybir
import concourse.tile as tile
from concourse.bass_utils import run_bass_kernel_spmd

N_POS = 131072
N_NEG = 393216
B = N_POS + N_NEG  # 524288
D = 256
NUM_REL = 500
NCORES = 8
SUB = 256          # rows per rel-pure sub-block
BS = 512           # rows per block (2 subs): DMA/TT1/moving granularity
GBLK = 32          # blocks per psum group

F8 = ml_dtypes.float8_e4m3
BF16 = ml_dtypes.bfloat16

f32 = mybir.dt.float32
bf16 = mybir.dt.bfloat16
fp8 = mybir.dt.float8e4
i32 = mybir.dt.int32
MULT = mybir.AluOpType.mult
MIN = mybir.AluOpType.min
MAX = mybir.AluOpType.max


# ---------------------------------------------------------------- host plan

def plan(rel_ids):
    """Sort rows by rel, pad groups to SUB multiples, split across cores."""
    rel_ids = np.asarray(rel_ids).astype(np.int64)
    order = np.argsort(rel_ids, kind="stable")
    counts = np.bincount(rel_ids, minlength=NUM_REL)
    subs_per_rel = (counts + SUB - 1) // SUB
    total_subs = int(subs_per_rel.sum())
    pad_subs = (-total_subs) % (2 * NCORES)
    s_all = total_subs + pad_subs

    starts = np.concatenate([[0], np.cumsum(counts)])
    pstarts = np.concatenate([[0], np.cumsum(subs_per_rel * SUB)])
    r_all = s_all * SUB
    padded_orig = np.full(r_all, -1, np.int64)
    sub_rel = np.zeros(s_all, np.int32)
    for k in range(NUM_REL):
        n = int(counts[k])
        if n == 0:
            continue
        p0 = int(pstarts[k])
        padded_orig[p0 : p0 + n] = order[starts[k] : starts[k] + n]
        sub_rel[p0 // SUB : p0 // SUB + int(subs_per_rel[k])] = k

    s_core = s_all // NCORES
    nblk = s_core // 2
    ngrp = (nblk + GBLK - 1) // GBLK
    return padded_orig, sub_rel, s_core, nblk, ngrp


# ---------------------------------------------------------------- device

def build_program(nblk, ngrp, debug=False):
    r_core = nblk * BS
    r_rect = ngrp * GBLK * BS

    nc = bacc.Bacc(
        "TRN2", target_bir_lowering=False, debug=False, num_devices=NCORES
    )
    npair = (ngrp + 1) // 2
    hT = nc.declare_dram_parameter("hT", [2, 128, r_core], fp8, isOutput=False)
    tT = nc.declare_dram_parameter("tT", [2, 128, r_core], fp8, isOutput=False)
    table = nc.declare_dram_parameter("table", [NUM_REL, D], bf16, isOutput=False)
    ids = nc.declare_dram_parameter("ids", [128, npair], i32, isOutput=False)
    ident4 = nc.declare_dram_parameter("ident4", [128, 128], bf16, isOutput=False)
    ub = nc.declare_dram_parameter("ub", [r_rect], f32, isOutput=False)
    lb = nc.declare_dram_parameter("lb", [r_rect], f32, isOutput=False)
    out = nc.declare_dram_parameter("out", [r_rect], f32, isOutput=True)
    if debug:
        dbg_st = nc.declare_dram_parameter("dbg_st", [128, 2, 2, GBLK * GBLK], f32, isOutput=True)
        dbg_q = nc.declare_dram_parameter("dbg_q", [128, 2, BS], f32, isOutput=True)
        dbg_s = nc.declare_dram_parameter("dbg_s", [GBLK, BS], f32, isOutput=True)
        dbg_rt = nc.declare_dram_parameter("dbg_rt", [128, D], f32, isOutput=True)

    hT_v = hT[:].rearrange("c p r -> p c r")
    tT_v = tT[:].rearrange("c p r -> p c r")
    # rect layouts indexed (j, g, c)
    ub_v = ub[:].rearrange("(j g c) -> j g c", j=GBLK, g=ngrp, c=BS)
    lb_v = lb[:].rearrange("(j g c) -> j g c", j=GBLK, g=ngrp, c=BS)
    out_v = out[:].rearrange("(j g c) -> j g c", j=GBLK, g=ngrp, c=BS)

    with tile.TileContext(nc) as tc:
        with (
            tc.tile_pool(name="io", bufs=1) as io,
            tc.tile_pool(name="hp", bufs=4) as hp,
            tc.tile_pool(name="tp", bufs=4) as tpo,
            tc.tile_pool(name="qp", bufs=4) as qp,
            tc.tile_pool(name="bp", bufs=4) as bp,
            tc.tile_pool(name="sp", bufs=4) as sp,
            tc.psum_pool(name="ps", bufs=2) as pspool,
            tc.psum_pool(name="tps", bufs=2) as tpspool,
        ):
            ids_t = io.tile([128, npair], i32)
            nc.gpsimd.dma_start(out=ids_t[:], in_=ids[:])
            ident_t = io.tile([128, 128], bf16)
            nc.sync.dma_start(out=ident_t[:], in_=ident4[:])

            # staging: [128, ch, u, slot*col] fp8; two fixed buffers
            stagings = [io.tile([128, 2, 2, GBLK * GBLK], fp8, name=f"staging{i}") for i in range(2)]
            for st in stagings:
                nc.vector.memset(st[:], 0.0)
            # pair gather: 128 rel rows; partition p = (g%2)*64 + u*32 + j
            reltiles = [io.tile([128, D], bf16, name=f"reltile{i}") for i in range(2)]

            def stage_pair(m):
                """Gather + stage groups 2m and 2m+1 (if present)."""
                rt = reltiles[m % 2]
                nc.gpsimd.indirect_dma_start(
                    out=rt[:],
                    out_offset=None,
                    in_=table[:],
                    in_offset=bass.IndirectOffsetOnAxis(
                        ap=ids_t[:, m : m + 1], axis=0
                    ),
                )
                for ch in range(2):
                    tpsum = tpspool.tile([128, 128], bf16, tag="tp")
                    nc.tensor.transpose(
                        tpsum[:], rt[:, ch * 128 : (ch + 1) * 128], ident_t[:]
                    )
                    for half in range(2):
                        g = 2 * m + half
                        if g >= ngrp:
                            break
                        st = stagings[g % 2]
                        for u in range(2):
                            base = half * 64 + u * 32
                            # scatter columns into diagonal slots (stride GBLK+1)
                            nc.vector.tensor_copy(
                                out=st[:, ch, u, 0 : GBLK * GBLK : GBLK + 1],
                                in_=tpsum[:, base : base + GBLK],
                            )

            stage_pair(0)

            for g in range(ngrp):
                nblk_g = min(GBLK, nblk - g * GBLK)
                st = stagings[g % 2]
                pss = [pspool.tile([GBLK, SUB], f32, tag=f"ps{u}", name=f"ps{u}") for u in range(2)]
                # DMA units of 4 blocks (2KB runs per partition/chunk)
                j = 0
                tt_unit = 0
                while j < nblk_g:
                    nu = min(4, nblk_g - j)
                    b0 = g * GBLK + j
                    ht = hp.tile([128, 2, 4 * BS], fp8, tag="h")
                    tt = tpo.tile([128, 2, 4 * BS], fp8, tag="t")
                    nc.sync.dma_start(
                        out=ht[:, :, : nu * BS], in_=hT_v[:, :, b0 * BS : (b0 + nu) * BS]
                    )
                    nc.scalar.dma_start(
                        out=tt[:, :, : nu * BS], in_=tT_v[:, :, b0 * BS : (b0 + nu) * BS]
                    )
                    qt = qp.tile([128, 2, 4 * BS], fp8, tag="q")
                    # TT1 in units of 2 blocks, split DVE/gpsimd
                    k = 0
                    while k < nu:
                        nk = min(2, nu - k)
                        eng = nc.gpsimd if (tt_unit % 8 in (3, 7)) else nc.vector
                        eng.tensor_tensor(
                            out=qt[:, :, k * BS : (k + nk) * BS],
                            in0=ht[:, :, k * BS : (k + nk) * BS],
                            in1=tt[:, :, k * BS : (k + nk) * BS],
                            op=MULT,
                        )
                        tt_unit += 1
                        k += nk
                    for k in range(nu):
                        jj = j + k
                        for u in range(2):
                            nc.tensor.matmul(
                                pss[u][:],
                                st[:, :, u, jj * GBLK : (jj + 1) * GBLK],
                                qt[:, :, k * BS + u * SUB : k * BS + (u + 1) * SUB],
                                start=(jj == 0),
                                stop=(jj == nblk_g - 1),
                                perf_mode=mybir.MatmulPerfMode.DoubleRow,
                            )
                    j += nu
                # tail for group g
                ubt = bp.tile([GBLK, BS], f32, tag="ub")
                lbt = bp.tile([GBLK, BS], f32, tag="lb")
                nc.sync.dma_start(out=ubt[:nblk_g], in_=ub_v[:nblk_g, g, :])
                nc.scalar.dma_start(out=lbt[:nblk_g], in_=lb_v[:nblk_g, g, :])
                c1 = sp.tile([GBLK, BS], f32, tag="c1")
                c2 = sp.tile([GBLK, BS], f32, tag="c2")
                for u in range(2):
                    cseg = slice(u * SUB, (u + 1) * SUB)
                    nc.vector.tensor_tensor(
                        out=c1[:nblk_g, cseg], in0=pss[u][:nblk_g],
                        in1=ubt[:nblk_g, cseg], op=MIN,
                    )
                nc.vector.tensor_tensor(
                    out=c2[:nblk_g], in0=c1[:nblk_g], in1=lbt[:nblk_g], op=MAX
                )
                sg = sp.tile([GBLK, BS], f32, tag="sg")
                nc.scalar.activation(
                    out=sg[:nblk_g], in_=c2[:nblk_g],
                    func=mybir.ActivationFunctionType.Sigmoid,
                )
                nc.scalar.dma_start(out=out_v[:nblk_g, g, :], in_=sg[:nblk_g])
                if debug and g == 0:
                    dst = sp.tile([128, 2, 2, GBLK * GBLK], f32, name="dst")
                    nc.vector.tensor_copy(out=dst[:], in_=st[:])
                    nc.sync.dma_start(out=dbg_st[:], in_=dst[:])
                    drt = sp.tile([128, D], f32, name="drt")
                    nc.vector.tensor_copy(out=drt[:], in_=reltiles[0][:])
                    nc.sync.dma_start(out=dbg_rt[:], in_=drt[:])
                    dsc = sp.tile([GBLK, BS], f32, name="dsc")
                    for u in range(2):
                        nc.vector.tensor_copy(out=dsc[:, u * SUB : (u + 1) * SUB], in_=pss[u][:])
                    nc.sync.dma_start(out=dbg_s[:], in_=dsc[:])
                if g % 2 == 1 and g + 1 < ngrp:
                    stage_pair((g + 1) // 2)

    nc.compile()
    return nc


# ---------------------------------------------------------------- host glue

def make_in_maps(inputs, padded_orig, sub_rel, s_core, nblk, ngrp):
    r_core = nblk * BS
    r_rect = ngrp * GBLK * BS

    head = np.asarray(inputs["head_embeds"], dtype=np.float32)
    tail = np.asarray(inputs["tail_embeds"], dtype=np.float32)
    lower = np.asarray(inputs["lower_bound"], dtype=np.float32)
    upper = np.asarray(inputs["upper_bound"], dtype=np.float32)
    table = np.asarray(inputs["rel_embeds"], dtype=np.float32).astype(BF16)

    h8 = head.astype(F8)
    t8 = tail.astype(F8)

    in_maps = []
    for c in range(NCORES):
        idx = padded_orig[c * r_core : (c + 1) * r_core]
        valid = idx >= 0
        hrows = np.zeros((r_core, D), dtype=F8)
        trows = np.zeros((r_core, D), dtype=F8)
        hrows[valid] = h8[idx[valid]]
        trows[valid] = t8[idx[valid]]
        hT = np.ascontiguousarray(hrows.reshape(r_core, 2, 128).transpose(1, 2, 0))
        tTm = np.ascontiguousarray(trows.reshape(r_core, 2, 128).transpose(1, 2, 0))

        ub_r = np.zeros(r_core, np.float32)
        lb_r = np.zeros(r_core, np.float32)
        mpos = valid & (idx < N_POS)
        mneg = idx >= N_POS
        ub_r[mpos] = upper[idx[mpos]]
        lb_r[mpos] = -np.inf
        ub_r[mneg] = np.inf
        lb_r[mneg] = lower[idx[mneg] - N_POS]

        def rectify(a):
            ap = np.zeros(r_rect, a.dtype)
            ap[:r_core] = a
            # (g, j, c) -> (j, g, c)
            return np.ascontiguousarray(
                ap.reshape(ngrp, GBLK, BS).transpose(1, 0, 2)
            ).reshape(-1)

        srel = sub_rel[c * s_core : (c + 1) * s_core]
        sp_ = np.zeros(ngrp * GBLK * 2, np.int32)
        sp_[:s_core] = srel
        srel_r = sp_.reshape(ngrp, GBLK, 2)  # (g, j, u)
        npair = (ngrp + 1) // 2
        ids_arr = np.zeros((128, npair), np.int32)
        for g in range(ngrp):
            # partition p = (g%2)*64 + u*32 + j
            ids_arr[(g % 2) * 64 : (g % 2) * 64 + 64, g // 2] = (
                srel_r[g].transpose(1, 0).reshape(64)
            )

        ident4 = np.eye(128, dtype=np.float32).astype(BF16)
        in_maps.append(
            {
                "hT": hT,
                "tT": tTm,
                "table": table,
                "ids": ids_arr,
                "ident4": ident4,
                "ub": rectify(ub_r),
                "lb": rectify(lb_r),
            }
        )
    return in_maps


def unpack_output(res, padded_orig, nblk, ngrp):
    r_core = nblk * BS
    out_full = np.empty(B, np.float32)
    for c in range(NCORES):
        rect = res.results[c]["out"].reshape(GBLK, ngrp, BS)
        flat = rect.transpose(1, 0, 2).reshape(-1)[:r_core]
        idx = padded_orig[c * r_core : (c + 1) * r_core]
        m = idx >= 0
        out_full[idx[m]] = flat[m]
    return out_full


def kernel(**inputs):
    padded_orig, sub_rel, s_core, nblk, ngrp = plan(inputs["rel_ids"])
    nc = build_program(nblk, ngrp)
    in_maps = make_in_maps(inputs, padded_orig, sub_rel, s_core, nblk, ngrp)
    res = run_bass_kernel_spmd(nc, in_maps, list(range(NCORES)))
    return unpack_output(res, padded_orig, nblk, ngrp)
